# revision 31
# baseline (speedup 1.0000x reference)
"""ColBERT-style max-sim retrieval kernel for 8 trn2 NeuronCores.

Computes, for query_h [Bq=128, Lq=32, H=256], doc_h [Bd=128, Ld=128, H=256],
W [256, 128], b [128]:

    q = l2norm(query_h @ W + b)          # [Bq, Lq, D=128]
    d = l2norm(doc_h  @ W + b)           # [Bd, Ld, D]
    logits[q, b] = sum_s max_t <q[q,s], d[b,t]>    # [Bq, Bd]

Sharding: docs split 8 x 16 across cores (queries replicated) -- minimizes
per-core HBM traffic (0.5MB/core doc slice + 4MB query vs 16MB full docs).
Each core computes a [128, 16] column block of the logits; host concatenates.

Per-core dataflow (all matmuls fp32r, 1 cycle/row on PE):
  - Host pre-transposes inputs to [H, tokens] so every matmul contracts over
    the partition dim with no on-device transposes.
  - Projection: psum_e[D=128p, tok] = W0.T@xT0 + W1.T@xT1 (PSUM accum).
  - Norm: ACT Square(psum_e + b) -> sq; PE ones[128,128].T @ sq gives the
    cross-partition sum-of-squares broadcast to all partitions in one matmul;
    rrep = ACT Abs_reciprocal_sqrt (same table set as Square/Copy -> a single
    ACT_TABLE_LOAD); normalize-multiply on DVE for doc chunks (fused
    scalar_tensor_tensor) and on ACT-copy + GPSIMD tensor_mul for query
    chunks, so during the score loop DVE does nothing but reduces.
  - Scores: per 128-query-token tile, lhsT = embq slice, rhs = embd (N=512
    matmuls) into PSUM [128, 8, 128]; DVE reduce_max over the innermost
    (doc-token) axis -- the structural bottleneck (67M score elements must
    each pass a 1-elem/lane/cycle DVE reduce; ~76us/core). The sum over
    query tokens is folded into PE via a sliding block-diagonal weight
    window accumulating into one PSUM bank. Score halves are split into two
    passes (docs 0-7 then 8-15) so reduces start after only ~1MB of input.
"""

import sys

import numpy as np

if "/opt/trn_rl_repo" not in sys.path:
    sys.path.insert(0, "/opt/trn_rl_repo")

import concourse.bass as bass
import concourse.tile as tile
from concourse import bacc, mybir
from concourse.bass_utils import run_bass_kernel_spmd

F32 = mybir.dt.float32
F32R = mybir.dt.float32r
AX = mybir.AxisListType
ALU = mybir.AluOpType
ACTF = mybir.ActivationFunctionType

# Problem constants (hardcoded per the harness contract).
BQ, LQ, BD, LD, H, D = 128, 32, 128, 128, 256, 128
NCORES = 8
DOCS_PER_CORE = BD // NCORES          # 16
NQ_TOK = BQ * LQ                      # 4096 query tokens (replicated)
ND_TOK = DOCS_PER_CORE * LD           # 2048 doc tokens per core
CHUNK = 512                           # embedding-phase token chunk (1 psum bank)
QS_TILES = NQ_TOK // 128              # 32 score row-tiles
GQ = 128 // LQ                        # 4 queries per qs-tile


def _build_program() -> bass.Bass:
    # Bacc (not plain Bass): its compile() runs move_matmul_waits_to_ldweights
    # and generate_event_semaphores, which split multi-wait matmuls into
    # event-semaphore helpers -- walrus rejects a fused matmul with >1 wait.
    nc = bacc.Bacc("TRN2", target_bir_lowering=False)

    qhT0 = nc.dram_tensor("qhT0", [128, NQ_TOK], F32R, kind="ExternalInput")
    qhT1 = nc.dram_tensor("qhT1", [128, NQ_TOK], F32R, kind="ExternalInput")
    dhT0 = nc.dram_tensor("dhT0", [128, ND_TOK], F32R, kind="ExternalInput")
    dhT1 = nc.dram_tensor("dhT1", [128, ND_TOK], F32R, kind="ExternalInput")
    # One packed constants tensor (one DMA): W0 | W1 | b | ones | Gpad
    NCONST = 128 + 128 + 1 + 128 + 256
    consts = nc.dram_tensor("consts", [128, NCONST], F32R, kind="ExternalInput")
    out_d = nc.dram_tensor("logits", [128, DOCS_PER_CORE], F32, kind="ExternalOutput")

    with tile.TileContext(nc) as tc:
        with (
            tc.tile_pool(name="consts", bufs=1) as constp,
            tc.tile_pool(name="inputs", bufs=1) as inp,
            tc.tile_pool(name="embs", bufs=1) as embp,
        ):
            # Constants in one DMA first (they gate the first matmuls), then
            # doc chunks split across both DMA queues (sync=HWDGE and
            # gpsimd=SWDGE run in parallel), then query chunks likewise.
            consts_sb = constp.tile([128, NCONST], F32R)
            nc.sync.dma_start(consts_sb[:], consts[:])
            w0_sb = consts_sb[:, 0:128]
            w1_sb = consts_sb[:, 128:256]
            b_sb = consts_sb[:, 256:257]
            ones_sb = consts_sb[:, 257:385]
            gpad_sb = consts_sb[:, 385:641]

            # Query chunk 0 rides at the head of both queues: the first
            # score reduces need it plus doc chunks 0-1, nothing else.
            dhT0_sb = inp.tile([128, ND_TOK], F32R)
            dhT1_sb = inp.tile([128, ND_TOK], F32R)
            qhT0_sb = inp.tile([128, NQ_TOK], F32R)
            qhT1_sb = inp.tile([128, NQ_TOK], F32R)
            nc.sync.dma_start(qhT0_sb[:, 0:CHUNK], qhT0[:, 0:CHUNK])
            nc.gpsimd.dma_start(qhT1_sb[:, 0:CHUNK], qhT1[:, 0:CHUNK])
            for c in range(0, ND_TOK, CHUNK):
                nc.sync.dma_start(dhT0_sb[:, c : c + CHUNK], dhT0[:, c : c + CHUNK])
                nc.gpsimd.dma_start(dhT1_sb[:, c : c + CHUNK], dhT1[:, c : c + CHUNK])
            for c in range(CHUNK, NQ_TOK, CHUNK):
                nc.sync.dma_start(qhT0_sb[:, c : c + CHUNK], qhT0[:, c : c + CHUNK])
                nc.sync.dma_start(qhT1_sb[:, c : c + CHUNK], qhT1[:, c : c + CHUNK])

            embq = embp.tile([128, NQ_TOK], F32R)   # normalized q emb [D, tok]
            embd = embp.tile([128, ND_TOK], F32R)   # normalized d emb [D, tok]

            # All pools coexist so query embedding chunks interleave with the
            # score loop (keeps DVE -- the bottleneck engine -- dense).
            # PSUM budget: pe 2 + ss 1 + sc 2x2 + logits 1 = 8 banks.
            with (
                tc.tile_pool(name="pe_psum", bufs=2, space="PSUM") as pep,
                tc.tile_pool(name="ss_psum", bufs=1, space="PSUM") as ssp,
                tc.tile_pool(name="sc_psum", bufs=2, space="PSUM") as scp,
                tc.tile_pool(name="lg_psum", bufs=1, space="PSUM") as lgp,
                tc.tile_pool(name="actwork", bufs=3) as actp,
                tc.tile_pool(name="maxv", bufs=4) as maxp,
                tc.tile_pool(name="outp", bufs=1) as outp,
            ):
                # The fused fp32r matmul (self-loading LDWEIGHTS) has a single
                # HW sync-wait slot, but matmuls whose operands arrive by DMA
                # on different semaphore lanes would need several waits and
                # walrus rejects them. Absorb each DMA wait with a tiny
                # self-referencing observer matmul (one wait each); after
                # these, PE's vector clock covers those DMA lanes.
                def pe_observe(x):
                    # N must be even for fp32r matmuls (ISA restriction);
                    # shares the pe-pool slots (transient, start of kernel).
                    ob = pep.tile([1, 2], F32, tag="pe")
                    nc.tensor.matmul(
                        ob[:], x[:, 0:1], x[:, 0:2], start=True, stop=True
                    )

                pe_observe(consts_sb)

                # Make the FIRST activation an Abs_reciprocal_sqrt so the
                # table-load pass picks abs_reciprocal_sqrt_and_small -- the
                # one set containing every function this kernel uses
                # (abs_reciprocal_sqrt, square, copy). Exactly one
                # ACT_TABLE_LOAD for the whole kernel.
                act_seed = actp.tile([128, 1], F32, tag="seed", bufs=1)
                nc.scalar.activation(
                    act_seed[:], ones_sb[:, 0:1], ACTF.Abs_reciprocal_sqrt
                )

                def emb_chunk(x0, x1, c, dst, on_gpsimd=False):
                    """Project+normalize tokens [c, c+CHUNK) of x into dst.

                    on_gpsimd: route the final normalize multiply through an
                    ACT copy + GPSIMD STT instead of a DVE STT. Used for the
                    query chunks that interleave with the score loop, keeping
                    DVE (the bottleneck) to pure reduce work there. The doc
                    chunks stay on DVE -- it is idle during the ramp anyway.
                    """
                    pe = pep.tile([128, CHUNK], F32, tag="pe")
                    nc.tensor.matmul(
                        pe[:], w0_sb[:], x0[:, c : c + CHUNK], start=True, stop=False
                    )
                    nc.tensor.matmul(
                        pe[:], w1_sb[:], x1[:, c : c + CHUNK], start=False, stop=True
                    )
                    # sq = (emb + b)^2  (bias fused into the activation)
                    sq = actp.tile([128, CHUNK], F32R, tag="sq")
                    nc.scalar.activation(sq[:], pe[:], ACTF.Square, bias=b_sb[:])
                    # Cross-partition sum of squares, broadcast to all
                    # partitions: ss[m, t] = sum_d sq[d, t] for every m.
                    ss = ssp.tile([128, CHUNK], F32, tag="ss")
                    nc.tensor.matmul(ss[:], ones_sb[:], sq[:], start=True, stop=True)
                    # rrep = 1/sqrt(|ss|); Abs_reciprocal_sqrt shares a table
                    # set with Square and Copy, so there is exactly one
                    # ACT_TABLE_LOAD in the whole kernel (Ln/Exp would thrash
                    # table sets against Square every chunk).
                    rrep = actp.tile([128, CHUNK], F32, tag="rrep")
                    nc.scalar.activation(rrep[:], ss[:], ACTF.Abs_reciprocal_sqrt)
                    # dst = (emb + b) * rrep
                    if on_gpsimd:
                        # Identity (unlike Copy) accepts a per-partition AP
                        # bias, so the +b rides on the PSUM->SBUF copy; Pool
                        # only supports plain tensor_tensor ops on trn2.
                        embb = actp.tile([128, CHUNK], F32, tag="embb")
                        nc.scalar.activation(
                            embb[:], pe[:], ACTF.Identity, bias=b_sb[:]
                        )
                        nc.gpsimd.tensor_mul(
                            dst[:, c : c + CHUNK], embb[:], rrep[:]
                        )
                    else:
                        nc.vector.scalar_tensor_tensor(
                            out=dst[:, c : c + CHUNK],
                            in0=pe[:],
                            scalar=b_sb[:],
                            in1=rrep[:],
                            op0=ALU.add,
                            op1=ALU.mult,
                        )

                logits_ps = lgp.tile([128, DOCS_PER_CORE], F32)
                # All 32 tiles' running maxes live in one persistent SBUF
                # buffer -- no pool recycling deps on the score stream.
                mvbuf = maxp.tile([128, QS_TILES, DOCS_PER_CORE], F32R)

                def score_half(i, h):
                    """Scores+max for qs-tile i, docs [8h, 8h+8)."""
                    qsl = embq[:, i * 128 : (i + 1) * 128]
                    sc = scp.tile([128, 8, 128], F32, tag="sc")
                    for j in range(2):
                        col = h * 1024 + j * 512
                        nc.tensor.matmul(
                            sc[:, j * 4 : (j + 1) * 4, :],
                            qsl,
                            embd[:, col : col + 512],
                            start=True,
                            stop=True,
                        )
                    nc.vector.reduce_max(
                        mvbuf[:, i, h * 8 : (h + 1) * 8], sc[:], axis=AX.X
                    )

                def group_sum(i):
                    # Accumulate sum over the 32 query tokens of each query via
                    # a sliding block-diagonal window of Gpad.
                    off = 124 - GQ * i
                    nc.tensor.matmul(
                        logits_ps[:],
                        gpad_sb[:, off : off + 128],
                        mvbuf[:, i, :],
                        start=(i == 0),
                        stop=(i == QS_TILES - 1),
                        skip_group_check=True,
                    )

                # Phase A: all h0 halves -- they only need doc chunks 0-1 and
                # the staggered query chunks, so the reduce stream starts as
                # soon as ~1MB of input has landed. Doc chunks 2-3 are
                # embedded concurrently (their DVE STTs slot into the score
                # stream); phase B (h1 halves + group sums) follows.
                emb_chunk(dhT0_sb, dhT1_sb, 0, embd)
                emb_chunk(qhT0_sb, qhT1_sb, 0, embq, on_gpsimd=True)
                emb_chunk(dhT0_sb, dhT1_sb, CHUNK, embd)
                score_half(0, 0)
                emb_chunk(dhT0_sb, dhT1_sb, 2 * CHUNK, embd)
                score_half(1, 0)
                emb_chunk(dhT0_sb, dhT1_sb, 3 * CHUNK, embd)
                for i in range(2, QS_TILES):
                    if i % 4 == 2 and (i // 4 + 1) * CHUNK < NQ_TOK:
                        emb_chunk(
                            qhT0_sb, qhT1_sb, (i // 4 + 1) * CHUNK, embq,
                            on_gpsimd=True,
                        )
                    score_half(i, 0)
                for i in range(QS_TILES):
                    score_half(i, 1)
                    group_sum(i)
                out_sb = outp.tile([128, DOCS_PER_CORE], F32)
                nc.scalar.copy(out_sb[:], logits_ps[:])
                nc.sync.dma_start(out_d[:], out_sb[:])

    nc.compile()
    return nc


def _host_inputs(query_h, doc_h, W, b):
    """Shard + lay out inputs for the 8 cores."""
    qT = np.ascontiguousarray(query_h.reshape(NQ_TOK, H).T)  # [256, 4096]
    gpad = np.zeros((128, 256), np.float32)
    for s in range(128):
        gpad[s, 124 + s // LQ] = 1.0
    consts = np.concatenate(
        [
            W[:128],
            W[128:],
            b.reshape(128, 1),
            np.ones((128, 128), np.float32),
            gpad,
        ],
        axis=1,
    )
    common = {
        "qhT0": np.ascontiguousarray(qT[:128]),
        "qhT1": np.ascontiguousarray(qT[128:]),
        "consts": np.ascontiguousarray(consts),
    }
    in_maps = []
    for k in range(NCORES):
        dT = np.ascontiguousarray(
            doc_h[k * DOCS_PER_CORE : (k + 1) * DOCS_PER_CORE].reshape(ND_TOK, H).T
        )
        in_maps.append(
            {
                **common,
                "dhT0": np.ascontiguousarray(dT[:128]),
                "dhT1": np.ascontiguousarray(dT[128:]),
            }
        )
    return in_maps


_PROGRAM = None


def _get_program() -> bass.Bass:
    global _PROGRAM
    if _PROGRAM is None:
        _PROGRAM = _build_program()
    return _PROGRAM


class _Runner:
    """Caches the sharded jitted executable so repeat calls skip rebuild.

    Mirrors bass2jax.run_bass_via_pjrt's multi-core branch: inputs for the 8
    cores are concatenated on axis 0 and shard_mapped over a 1-D core mesh,
    with pre-zeroed donated output buffers.
    """

    def __init__(self):
        import jax
        import numpy as _np
        from jax.sharding import Mesh, PartitionSpec
        from jax.experimental.shard_map import shard_map
        from concourse import bass2jax, mybir as _mb

        bass2jax.install_neuronx_cc_hook()
        nc = _get_program()
        self.nc = nc

        partition_name = (
            nc.partition_id_tensor.name if nc.partition_id_tensor else None
        )
        in_names, out_names, out_avals, zero_outs = [], [], [], []
        for alloc in nc.m.functions[0].allocations:
            if not isinstance(alloc, _mb.MemoryLocationSet):
                continue
            name = alloc.memorylocations[0].name
            if alloc.kind == "ExternalInput":
                if name != partition_name:
                    in_names.append(name)
            elif alloc.kind == "ExternalOutput":
                shape = tuple(alloc.tensor_shape)
                dt_np = _mb.dt.np(alloc.dtype)
                out_names.append(name)
                out_avals.append(jax.core.ShapedArray(shape, dt_np))
                zero_outs.append(_np.zeros(shape, dt_np))

        n_params = len(in_names)
        n_outs = len(out_names)
        all_in_names = list(in_names) + list(out_names)
        if partition_name is not None:
            all_in_names.append(partition_name)

        def _body(*args):
            operands = list(args)
            if partition_name is not None:
                operands.append(bass2jax.partition_id_tensor())
            outs = bass2jax._bass_exec_p.bind(
                *operands,
                out_avals=tuple(out_avals),
                in_names=tuple(all_in_names),
                out_names=tuple(out_names),
                lowering_input_output_aliases=(),
                sim_require_finite=True,
                sim_require_nnan=True,
                nc=nc,
            )
            return tuple(outs)

        devices = jax.devices()[:NCORES]
        mesh = Mesh(np.asarray(devices), ("core",))
        in_specs = (PartitionSpec("core"),) * (n_params + n_outs)
        out_specs = (PartitionSpec("core"),) * n_outs
        self._fn = jax.jit(
            shard_map(
                _body,
                mesh=mesh,
                in_specs=in_specs,
                out_specs=out_specs,
                check_rep=False,
            ),
            donate_argnums=tuple(range(n_params, n_params + n_outs)),
            keep_unused=True,
        )
        self.in_names = in_names
        self.out_names = out_names
        self.out_avals = out_avals
        self.zero_outs = zero_outs
        self.n_params = n_params

    def concat_inputs(self, in_maps):
        return [
            np.concatenate([np.asarray(m[name]) for m in in_maps], axis=0)
            for name in self.in_names
        ]

    def concat_zeros(self):
        return [
            np.zeros((NCORES * z.shape[0], *z.shape[1:]), z.dtype)
            for z in self.zero_outs
        ]

    def run(self, concat_in):
        out_arrs = self._fn(*concat_in, *self.concat_zeros())
        return out_arrs

    def results(self, out_arrs):
        return [
            {
                name: np.asarray(out_arrs[i]).reshape(
                    NCORES, *self.out_avals[i].shape
                )[c]
                for i, name in enumerate(self.out_names)
            }
            for c in range(NCORES)
        ]


_RUNNER = None


def _get_runner() -> "_Runner":
    global _RUNNER
    if _RUNNER is None:
        _RUNNER = _Runner()
    return _RUNNER


def kernel(query_h, doc_h, W, b):
    query_h = np.asarray(query_h, np.float32)
    doc_h = np.asarray(doc_h, np.float32)
    W = np.asarray(W, np.float32)
    b = np.asarray(b, np.float32)

    in_maps = _host_inputs(query_h, doc_h, W, b)
    runner = _get_runner()
    outs = runner.results(runner.run(runner.concat_inputs(in_maps)))
    return np.concatenate(
        [outs[k]["logits"] for k in range(NCORES)], axis=1
    ).astype(np.float32)


def bench(query_h, doc_h, W, b, iters=20):
    """Repeat-execute timing with device-resident inputs. Returns times (s)."""
    import time
    import jax

    in_maps = _host_inputs(
        np.asarray(query_h, np.float32),
        np.asarray(doc_h, np.float32),
        np.asarray(W, np.float32),
        np.asarray(b, np.float32),
    )
    runner = _get_runner()
    concat_in = [jax.device_put(a) for a in runner.concat_inputs(in_maps)]
    # warmup (also triggers compile)
    jax.block_until_ready(runner.run(concat_in))
    times = []
    for _ in range(iters):
        t0 = time.perf_counter()
        jax.block_until_ready(runner.run(concat_in))
        times.append(time.perf_counter() - t0)
    return times
